# revision 35
# baseline (speedup 1.0000x reference)
"""Decagon GNN message-passing kernel for 8 Trainium2 NeuronCores.

Strategy (SPMD, no collectives, no dma_gather):
  - Only edges with dst < nD=1000 affect the output (finalX = x[:nD]).  The
    small GNN (encoder MLP, edge aggregation, SAGE layer) is REPLICATED.
  - Edge aggregation runs as a dense matmul against a host-built adjacency
    matrix: meant^T = sum_sw h2n_sw^T @ (A*rr)_sw + (aggprot*rr)^T, where the
    protein-source contribution (input-only: proteinEmb rows summed by
    edge_index) and the 1/max(cnt,1) scaling fold in on the host.
  - The decoder exploits out[t] = U[a_t] + V[b_t] + B with U = finalX @ Wfa,
    V = finalX @ Wfb ([nD, 500] each, computed on device).  Pairs are sharded
    across cores and host-sorted into 64 (a>>7, b>>7) buckets, so each <=512
    pair chunk is TWO matmuls: po[se,t] = U_wa[:,se]^T @ Ma + V_wb[:,se]^T @ Mb
    with host-built one-hot Ma/Mb streamed from SBUF.
  - Output is written transposed ([500, Tc] per core) in bf16 via large staged
    DMAs; the host unsorts/transposes/casts.
  - hardshrink (lambda=1e-6) is numerically an identity at fp32 scale; the two
    decoder matmuls fuse on the host: Wf = outW1 @ outW2[:, samp].
"""

import os

import numpy as np
import ml_dtypes

import concourse.bass as bass
import concourse.tile as tile
from concourse import bacc, mybir

BF16 = mybir.dt.bfloat16
F32 = mybir.dt.float32
FP8 = mybir.dt.float8e4

P = 128
D = 128
ND = 1000
NDP = 1024            # ND padded to 8 windows of 128
NW = 8
F = 2048
KF = F // P           # 16 k-tiles over feature dim
NCORES = 8
T = 150000
TCPP = 19072          # packed per-core columns (sum of align-4 bucket pads)
QW = TCPP // 4        # M upload quarter width (decoder starts on quarter 0)
SOUT = 500            # sampled output columns
MT = 4
MROW = 125            # 4 x 125 output-row tiles
NCHMAX = 96           # static upper bound on per-core chunk count


def _bf16(x):
    return np.asarray(x, dtype=np.float32).astype(ml_dtypes.bfloat16)


def _build_program(chunks):
    """Emit the SPMD bass program.  chunks: list of (wa, wb, off, n) decoder
    chunk descriptors (identical structure across cores; off/n are the packed
    column ranges inside ma/mb/out)."""
    nc = bacc.Bacc("TRN2", target_bir_lowering=False)

    # ---- I/O ----
    dft = nc.declare_dram_parameter("dft", [P, KF, ND], BF16, isOutput=False)
    w1l = nc.declare_dram_parameter("w1l", [P, KF, P], BF16, isOutput=False)
    w2 = nc.declare_dram_parameter("w2", [P, P], BF16, isOutput=False)
    wl = nc.declare_dram_parameter("wl", [P, P], BF16, isOutput=False)
    wr = nc.declare_dram_parameter("wr", [P, P], BF16, isOutput=False)
    wfa = nc.declare_dram_parameter("wfa", [P, SOUT], BF16, isOutput=False)
    wfb = nc.declare_dram_parameter("wfb", [P, SOUT], BF16, isOutput=False)
    b1c = nc.declare_dram_parameter("b1c", [P, 1], F32, isOutput=False)
    b2c = nc.declare_dram_parameter("b2c", [P, 1], F32, isOutput=False)
    blc = nc.declare_dram_parameter("blc", [P, 1], F32, isOutput=False)
    b2fr = nc.declare_dram_parameter("b2fr", [1, SOUT], F32, isOutput=False)
    ones1 = nc.declare_dram_parameter("ones1", [1, P], F32, isOutput=False)
    idn = nc.declare_dram_parameter("idn", [P, P], BF16, isOutput=False)
    amat = nc.declare_dram_parameter("amat", [P, NW, NDP], BF16, isOutput=False)
    apm = nc.declare_dram_parameter("apm", [P, NW, P], BF16, isOutput=False)
    ma = nc.declare_dram_parameter("ma", [P, TCPP], BF16, isOutput=False)
    mb = nc.declare_dram_parameter("mb", [P, TCPP], BF16, isOutput=False)
    out = nc.declare_dram_parameter("out", [MT, MROW, TCPP], BF16, isOutput=True)

    with tile.TileContext(nc) as tc:
        with tc.tile_pool(name="const", bufs=1) as const, \
             tc.tile_pool(name="persist", bufs=1) as persist:

            # encoder inputs first (they gate the critical path) ...
            aggp = tc.alloc_tile_pool(name="aggc", bufs=1)
            encp = tc.alloc_tile_pool(name="enc", bufs=1)
            dft_sb = []
            for j in range(8):
                t2 = encp.tile([P, 2, ND], BF16, tag=f"dft{j}")
                nc.sync.dma_start(t2[:], dft[:, 2 * j : 2 * j + 2, :])
                dft_sb.append(t2)
            w1l_sb = encp.tile([P, KF, P], BF16)
            nc.sync.dma_start(w1l_sb[:], w1l[:, :, :])

            # ... then the rest (overlaps encoder compute)
            w2_sb = const.tile([P, P], BF16)
            nc.sync.dma_start(w2_sb[:], w2[:, :])
            wl_sb = const.tile([P, P], BF16)
            nc.sync.dma_start(wl_sb[:], wl[:, :])
            wr_sb = const.tile([P, P], BF16)
            nc.sync.dma_start(wr_sb[:], wr[:, :])
            wfa_sb = const.tile([P, SOUT], BF16)
            nc.sync.dma_start(wfa_sb[:], wfa[:, :])
            wfb_sb = const.tile([P, SOUT], BF16)
            nc.sync.dma_start(wfb_sb[:], wfb[:, :])
            b1c_sb = const.tile([P, 1], F32)
            nc.sync.dma_start(b1c_sb[:], b1c[:, :])
            b2c_sb = const.tile([P, 1], F32)
            nc.sync.dma_start(b2c_sb[:], b2c[:, :])
            blc_sb = const.tile([P, 1], F32)
            nc.sync.dma_start(blc_sb[:], blc[:, :])
            b2fr_sb = const.tile([1, SOUT], F32)
            nc.sync.dma_start(b2fr_sb[:], b2fr[:, :])
            ones1_sb = const.tile([1, P], F32)
            nc.sync.dma_start(ones1_sb[:], ones1[:, :])
            idn_sb = const.tile([P, P], BF16)
            nc.sync.dma_start(idn_sb[:], idn[:, :])
            amat_sb = aggp.tile([P, NW, NDP], BF16)
            nc.sync.dma_start(amat_sb[:], amat[:, :, :])
            apm_sb = aggp.tile([P, NW, P], BF16)
            nc.sync.dma_start(apm_sb[:], apm[:, :, :])
            ma_sb, mb_sb = [], []
            for q in range(4):
                ta = const.tile([P, QW], BF16, tag=f"maq{q}")
                nc.sync.dma_start(ta[:], ma[:, q * QW : (q + 1) * QW])
                ma_sb.append(ta)
                tb = const.tile([P, QW], BF16, tag=f"mbq{q}")
                nc.sync.dma_start(tb[:], mb[:, q * QW : (q + 1) * QW])
                mb_sb.append(tb)

            h2t = persist.tile([P, NW * P], BF16)   # encoder out [d, node]
            h2n = persist.tile([P, NW, P], BF16)    # transposed   [node, d]
            xt = persist.tile([P, NW, P], BF16)     # finalX       [d, node]
            u_sb = persist.tile([P, NW, SOUT], BF16)  # U = finalX@Wfa [node, se]
            v_sb = persist.tile([P, NW, SOUT], BF16)  # V = finalX@Wfb [node, se]

            # ---- phase 1: encoder MLP (replicated) ----
            with tc.tile_pool(name="encps", bufs=2, space=bass.MemorySpace.PSUM) as encps, \
                 tc.tile_pool(name="trps", bufs=2, space=bass.MemorySpace.PSUM) as trps:
                h1t = encp.tile([P, ND], BF16)
                nc.vector.memset(h2t[:, ND:], 0.0)
                for c0, cw in ((0, 512), (512, ND - 512)):
                    ph = encps.tile([P, 512], F32, tag="ph")
                    for k in range(KF):
                        nc.tensor.matmul(
                            ph[:, :cw],
                            w1l_sb[:, k, :],
                            dft_sb[k // 2][:, k % 2, c0 : c0 + cw],
                            start=(k == 0),
                            stop=(k == KF - 1),
                        )
                    nc.scalar.activation(
                        h1t[:, c0 : c0 + cw], ph[:, :cw],
                        mybir.ActivationFunctionType.Relu, bias=b1c_sb[:],
                    )
                for c0, cw in ((0, 512), (512, ND - 512)):
                    ph = encps.tile([P, 512], F32, tag="ph")
                    nc.tensor.matmul(ph[:, :cw], w2_sb[:], h1t[:, c0 : c0 + cw])
                    nc.scalar.activation(
                        h2t[:, c0 : c0 + cw], ph[:, :cw],
                        mybir.ActivationFunctionType.Relu, bias=b2c_sb[:],
                    )
                # h2 windows transposed to [node, d] (lhsT for aggregation)
                for w in range(NW):
                    pt = trps.tile([P, P], BF16, tag="pt")
                    nc.tensor.transpose(pt[:], h2t[:, w * P : (w + 1) * P], idn_sb[:])
                    nc.scalar.copy(h2n[:, w, :], pt[:])
            encp.release()

            # ---- phase 2: aggregation + SAGE + U/V (replicated) ----
            with tc.tile_pool(name="gnn", bufs=2) as gnnp, \
                 tc.tile_pool(name="aggps", bufs=2, space=bass.MemorySpace.PSUM) as aggps, \
                 tc.tile_pool(name="smps", bufs=2, space=bass.MemorySpace.PSUM) as smps, \
                 tc.tile_pool(name="uvps", bufs=1, space=bass.MemorySpace.PSUM) as uvps:
                # decoder bias folded into U: every pair reads exactly one U
                # row, so U += B (broadcast to all node rows) replaces the
                # per-chunk bias add in the decoder copies
                bbc = gnnp.tile([P, SOUT], BF16, tag="bbc")
                pb = uvps.tile([P, SOUT], F32, tag="pb")
                nc.tensor.matmul(pb[:], ones1_sb[:], b2fr_sb[:])
                nc.scalar.copy(bbc[:], pb[:])
                for w in range(NW):
                    pagg = aggps.tile([P, P], F32, tag="pagg")
                    for sw in range(NW):
                        nc.tensor.matmul(
                            pagg[:], h2n[:, sw, :],
                            amat_sb[:, sw, w * P : (w + 1) * P],
                            start=(sw == 0), stop=False,
                        )
                    nc.tensor.matmul(pagg[:], apm_sb[:, w, :], idn_sb[:],
                                     start=False, stop=True)
                    meant = gnnp.tile([P, P], BF16, tag="meant")
                    nc.scalar.copy(meant[:], pagg[:])

                    px = smps.tile([P, P], F32, tag="px")
                    nc.tensor.matmul(px[:], wl_sb[:], meant[:], start=True, stop=False)
                    nc.tensor.matmul(px[:], wr_sb[:], h2t[:, w * P : (w + 1) * P],
                                     start=False, stop=True)
                    nc.scalar.activation(
                        xt[:, w, :], px[:],
                        mybir.ActivationFunctionType.Relu, bias=blc_sb[:],
                    )
                    pu = uvps.tile([P, SOUT], F32, tag="pu")
                    nc.tensor.matmul(pu[:], xt[:, w, :], wfa_sb[:])
                    nc.vector.tensor_tensor(
                        u_sb[:, w, :], pu[:], bbc[:], mybir.AluOpType.add
                    )
                    pv = uvps.tile([P, SOUT], F32, tag="pv")
                    nc.tensor.matmul(pv[:], xt[:, w, :], wfb_sb[:])
                    nc.vector.tensor_copy(v_sb[:, w, :], pv[:])
            aggp.release()

            # ---- phase 3: decoder (sharded over cores) ----
            with tc.tile_pool(name="dec", bufs=2) as decp, \
                 tc.tile_pool(name="decps", bufs=4, space=bass.MemorySpace.PSUM) as decps:
                pe = max(o + n for _, _, o, n in chunks)
                for mt in range(MT):
                    stage = decp.tile([P, TCPP], BF16, tag="stage")
                    sent = 0
                    for ci, (wa, wb, off, n) in enumerate(chunks):
                        q, lo = off // QW, off % QW
                        assert lo + n <= QW
                        po = decps.tile([P, 512], F32, tag="po")
                        nc.tensor.matmul(
                            po[:MROW, :n],
                            u_sb[:, wa, mt * MROW : (mt + 1) * MROW],
                            ma_sb[q][:, lo : lo + n],
                            start=True, stop=False,
                        )
                        nc.tensor.matmul(
                            po[:MROW, :n],
                            v_sb[:, wb, mt * MROW : (mt + 1) * MROW],
                            mb_sb[q][:, lo : lo + n],
                            start=False, stop=True,
                        )
                        if ci % 2 == 0:
                            nc.vector.tensor_copy(
                                stage[:MROW, off : off + n], po[:MROW, :n]
                            )
                        else:
                            nc.scalar.copy(
                                stage[:MROW, off : off + n], po[:MROW, :n]
                            )
                        # flush each completed quarter with a large DMA,
                        # alternating SWDGE (spreads over all 16 SDMA
                        # engines) and HWDGE (5 engines, otherwise idle here)
                        end = off + n
                        if end == pe or end % QW == 0:
                            eng = nc.gpsimd if (mt + end // QW) % 2 else nc.sync
                            eng.dma_start(out[mt, :, sent:end],
                                          stage[:MROW, sent:end])
                            sent = end

    nc.compile()
    return nc


def _prepare(inputs):
    """Host-side preprocessing: weight fusion/casts, adjacency + protein
    aggregation matrices, per-core sorted pair one-hots."""
    dF = np.asarray(inputs["drugFeatures"], np.float32)
    ei = np.asarray(inputs["edge_index"])
    tpl = np.asarray(inputs["tpl"])
    samp = np.asarray(inputs["sampleSes"]).astype(np.int64)
    W1 = np.asarray(inputs["W1"], np.float32)
    b1 = np.asarray(inputs["b1"], np.float32)
    W2 = np.asarray(inputs["W2"], np.float32)
    b2 = np.asarray(inputs["b2"], np.float32)
    prot = np.asarray(inputs["proteinEmb"], np.float32)
    sageWl = np.asarray(inputs["sageWl"], np.float32)
    sageBl = np.asarray(inputs["sageBl"], np.float32)
    sageWr = np.asarray(inputs["sageWr"], np.float32)
    outW1 = np.asarray(inputs["outW1"], np.float32)
    outB1 = np.asarray(inputs["outB1"], np.float32)
    outW2 = np.asarray(inputs["outW2"], np.float32)
    outB2 = np.asarray(inputs["outB2"], np.float32)

    # ---- edges: keep dst < ND; fold 1/max(cnt,1) into A and aggprot ----
    src = ei[0].astype(np.int64)
    dst = ei[1].astype(np.int64)
    keep = dst < ND
    src = src[keep]
    dst = dst[keep]
    cnt = np.bincount(dst, minlength=NDP).astype(np.float32)[:NDP]
    rr = 1.0 / np.maximum(cnt, 1.0)

    isdrug = src < ND
    sd, dd = src[isdrug], dst[isdrug]
    A = np.zeros((NDP, NDP), np.float32)          # [src, dst] edge counts
    np.add.at(A, (sd, dd), 1.0)
    A *= rr[None, :]
    amat = _bf16(A.reshape(NW, P, NDP).transpose(1, 0, 2))

    sp, dp = src[~isdrug] - ND, dst[~isdrug]
    ap = np.zeros((NDP, D), np.float32)           # [dst, d] protein sums
    np.add.at(ap, dp, prot[sp])
    ap *= rr[:, None]
    apm = _bf16(ap.reshape(NW, P, D).transpose(1, 0, 2))

    # ---- fused decoder weights ----
    W2s = outW2[:, samp]                     # [128, 500]
    Wf = outW1 @ W2s                         # [256, 500]
    B2f = outB1 @ W2s + outB2[samp]          # [500]

    common = dict(
        dft=_bf16(dF.T.reshape(KF, P, ND).transpose(1, 0, 2)),
        w1l=_bf16(W1.reshape(KF, P, P).transpose(1, 0, 2)),
        w2=_bf16(W2),
        wl=_bf16(sageWl),
        wr=_bf16(sageWr),
        wfa=_bf16(Wf[:P, :]),
        wfb=_bf16(Wf[P:, :]),
        b1c=b1.reshape(P, 1).astype(np.float32),
        b2c=b2.reshape(P, 1).astype(np.float32),
        blc=sageBl.reshape(P, 1).astype(np.float32),
        b2fr=B2f.reshape(1, SOUT).astype(np.float32),
        ones1=np.ones((1, P), np.float32),
        idn=_bf16(np.eye(P)),
        amat=amat,
        apm=apm,
    )

    # ---- global pair sort into 64 (wa, wb) buckets, dealt across cores ----
    # All T pairs sort by bucket; within a bucket, element r goes to core
    # r % 8 at packed column offs[bucket] + r // 8.  Per-core bucket sizes
    # then differ by <=1, so one shared chunk layout serves every core.
    a_all = tpl[:, 0].astype(np.int64)
    b_all = tpl[:, 1].astype(np.int64)
    key = (a_all >> 7) * NW + (b_all >> 7)
    order = np.argsort(key, kind="stable")
    ksort = key[order]
    sizes = np.bincount(ksort, minlength=NW * NW)
    starts = np.cumsum(sizes) - sizes
    rank = np.arange(T) - np.repeat(starts, sizes)
    core = rank % NCORES
    bucket_pad = ((sizes + NCORES - 1) // NCORES + 3) // 4 * 4
    offs = np.zeros(NW * NW + 1, np.int64)
    offs[1:] = np.cumsum(bucket_pad)
    assert offs[-1] <= TCPP, (offs[-1], TCPP)
    col = offs[ksort] + rank // NCORES
    chunks = []
    for bi in range(NW * NW):
        wa, wb = bi // NW, bi % NW
        o, n = int(offs[bi]), int(bucket_pad[bi])
        while n:
            # split at 512 and at M-upload quarter boundaries
            step = min(n, 512, QW - o % QW)
            chunks.append((wa, wb, o, step))
            o += step
            n -= step
    assert len(chunks) <= NCHMAX, len(chunks)

    a_sorted = a_all[order]
    b_sorted = b_all[order]
    gcols = np.empty(T, np.int64)
    gcols[order] = core * TCPP + col
    ma_list, mb_list = [], []
    for c in range(NCORES):
        m = core == c
        Ma = np.zeros((P, TCPP), ml_dtypes.bfloat16)
        Mb = np.zeros((P, TCPP), ml_dtypes.bfloat16)
        Ma[a_sorted[m] & 127, col[m]] = 1.0
        Mb[b_sorted[m] & 127, col[m]] = 1.0
        ma_list.append(Ma)
        mb_list.append(Mb)
    return common, chunks, ma_list, mb_list, gcols


LAST_RESULTS = None


def kernel(**inputs):
    common, chunks, ma_list, mb_list, gcols = _prepare(inputs)
    nc = _build_program(chunks)

    in_maps = [
        {**common, "ma": ma_list[c], "mb": mb_list[c]} for c in range(NCORES)
    ]

    if os.environ.get("BASS_SIM"):
        from concourse.bass_interp import CoreSim

        sim = CoreSim(nc)
        for k, v in in_maps[0].items():
            sim.tensor(k)[:] = v
        sim.simulate()
        outs = [np.array(sim.tensor("out"))]
        outs = outs * NCORES  # sim runs core 0 only; replicate for shape
    else:
        from concourse.bass_utils import run_bass_kernel_spmd

        res = run_bass_kernel_spmd(
            nc, in_maps, list(range(NCORES)),
            tmpdir=os.environ.get("BASS_TRACE_DIR") or None,
        )
        global LAST_RESULTS
        LAST_RESULTS = res
        outs = [res.results[c]["out"] for c in range(NCORES)]

    full = np.concatenate(
        [np.asarray(o).reshape(SOUT, TCPP) for o in outs], axis=1
    )  # [500, 8*TCPP] bf16, packed-sorted columns
    fullT = np.ascontiguousarray(full.T)
    return fullT[gcols].astype(np.float32).reshape(-1)


if __name__ == "__main__":
    rng = np.random.default_rng(0)
    fake = dict(
        drugFeatures=rng.standard_normal((ND, F), dtype=np.float32),
        edge_index=rng.integers(0, 21000, (2, 640000)),
        tpl=rng.integers(0, ND, (T, 2)),
        sampleSes=rng.integers(0, 964, (SOUT,)),
        W1=rng.standard_normal((F, D), dtype=np.float32) * 0.02,
        b1=np.zeros(D, np.float32),
        W2=rng.standard_normal((D, D), dtype=np.float32) * 0.05,
        b2=np.zeros(D, np.float32),
        proteinEmb=rng.uniform(0.001, 0.3, (20000, D)).astype(np.float32),
        sageWl=rng.standard_normal((D, D), dtype=np.float32) * 0.05,
        sageBl=np.zeros(D, np.float32),
        sageWr=rng.standard_normal((D, D), dtype=np.float32) * 0.05,
        outW1=rng.standard_normal((2 * D, D), dtype=np.float32) * 0.05,
        outB1=np.zeros(D, np.float32),
        outW2=rng.standard_normal((D, 964), dtype=np.float32) * 0.05,
        outB2=np.zeros(964, np.float32),
    )
    out = kernel(**fake)
    print(out.shape, out.dtype)


# revision 36
# speedup vs baseline: 1.3028x; 1.3028x over previous
"""Decagon GNN message-passing kernel for 8 Trainium2 NeuronCores.

Strategy (SPMD, no collectives, no dma_gather):
  - Only edges with dst < nD=1000 affect the output (finalX = x[:nD]).  The
    small GNN (encoder MLP, edge aggregation, SAGE layer) is REPLICATED.
  - Edge aggregation runs as a dense matmul against a host-built adjacency
    matrix: meant^T = sum_sw h2n_sw^T @ (A*rr)_sw + (aggprot*rr)^T, where the
    protein-source contribution (input-only: proteinEmb rows summed by
    edge_index) and the 1/max(cnt,1) scaling fold in on the host.
  - The decoder exploits out[t] = U[a_t] + V[b_t] + B with U = finalX @ Wfa,
    V = finalX @ Wfb ([nD, 500] each, computed on device).  Pairs are sharded
    across cores and host-sorted into 64 (a>>7, b>>7) buckets, so each <=512
    pair chunk is TWO matmuls: po[se,t] = U_wa[:,se]^T @ Ma + V_wb[:,se]^T @ Mb
    with host-built one-hot Ma/Mb streamed from SBUF.
  - Output is written transposed ([500, Tc] per core) in bf16 via large staged
    DMAs; the host unsorts/transposes/casts.
  - hardshrink (lambda=1e-6) is numerically an identity at fp32 scale; the two
    decoder matmuls fuse on the host: Wf = outW1 @ outW2[:, samp].
"""

import os

import numpy as np
import ml_dtypes

import concourse.bass as bass
import concourse.tile as tile
from concourse import bacc, mybir

BF16 = mybir.dt.bfloat16
F32 = mybir.dt.float32
FP8 = mybir.dt.float8e4

P = 128
D = 128
ND = 1000
NDP = 1024            # ND padded to 8 windows of 128
NW = 8
F = 2048
KF = F // P           # 16 k-tiles over feature dim
NCORES = 8
T = 150000
TCPP = 19072          # packed per-core columns (sum of align-4 bucket pads)
QW = TCPP // 4        # M upload quarter width (decoder starts on quarter 0)
SOUT = 500            # sampled output columns
MT = 4
MROW = 125            # 4 x 125 output-row tiles
NCHMAX = 96           # static upper bound on per-core chunk count


def _bf16(x):
    return np.asarray(x, dtype=np.float32).astype(ml_dtypes.bfloat16)


def _build_program(chunks):
    """Emit the SPMD bass program.  chunks: list of (wa, wb, off, n) decoder
    chunk descriptors (identical structure across cores; off/n are the packed
    column ranges inside ma/mb/out)."""
    nc = bacc.Bacc("TRN2", target_bir_lowering=False)

    # ---- I/O ----
    dft = nc.declare_dram_parameter("dft", [P, KF, ND], BF16, isOutput=False)
    w1l = nc.declare_dram_parameter("w1l", [P, KF, P], BF16, isOutput=False)
    w2 = nc.declare_dram_parameter("w2", [P, P], BF16, isOutput=False)
    wl = nc.declare_dram_parameter("wl", [P, P], BF16, isOutput=False)
    wr = nc.declare_dram_parameter("wr", [P, P], BF16, isOutput=False)
    wfa = nc.declare_dram_parameter("wfa", [P, SOUT], BF16, isOutput=False)
    wfb = nc.declare_dram_parameter("wfb", [P, SOUT], BF16, isOutput=False)
    b1c = nc.declare_dram_parameter("b1c", [P, 1], F32, isOutput=False)
    b2c = nc.declare_dram_parameter("b2c", [P, 1], F32, isOutput=False)
    blc = nc.declare_dram_parameter("blc", [P, 1], F32, isOutput=False)
    b2fr = nc.declare_dram_parameter("b2fr", [1, SOUT], F32, isOutput=False)
    ones1 = nc.declare_dram_parameter("ones1", [1, P], F32, isOutput=False)
    idn = nc.declare_dram_parameter("idn", [P, P], BF16, isOutput=False)
    amat = nc.declare_dram_parameter("amat", [P, NW, NDP], BF16, isOutput=False)
    apm = nc.declare_dram_parameter("apm", [P, NW, P], BF16, isOutput=False)
    ma = nc.declare_dram_parameter("ma", [P, TCPP], BF16, isOutput=False)
    mb = nc.declare_dram_parameter("mb", [P, TCPP], BF16, isOutput=False)
    out = nc.declare_dram_parameter("out", [MT, MROW, TCPP], BF16, isOutput=True)

    with tile.TileContext(nc) as tc:
        with tc.tile_pool(name="const", bufs=1) as const, \
             tc.tile_pool(name="persist", bufs=1) as persist:

            # encoder inputs first (they gate the critical path) ...
            aggp = tc.alloc_tile_pool(name="aggc", bufs=1)
            encp = tc.alloc_tile_pool(name="enc", bufs=1)
            dft_sb = []
            for j in range(8):
                t2 = encp.tile([P, 2, ND], BF16, tag=f"dft{j}")
                nc.sync.dma_start(t2[:], dft[:, 2 * j : 2 * j + 2, :])
                dft_sb.append(t2)
            w1l_sb = encp.tile([P, KF, P], BF16)
            nc.sync.dma_start(w1l_sb[:], w1l[:, :, :])

            # ... then the rest (overlaps encoder compute)
            w2_sb = const.tile([P, P], BF16)
            nc.sync.dma_start(w2_sb[:], w2[:, :])
            wl_sb = const.tile([P, P], BF16)
            nc.sync.dma_start(wl_sb[:], wl[:, :])
            wr_sb = const.tile([P, P], BF16)
            nc.sync.dma_start(wr_sb[:], wr[:, :])
            wfa_sb = const.tile([P, SOUT], BF16)
            nc.sync.dma_start(wfa_sb[:], wfa[:, :])
            wfb_sb = const.tile([P, SOUT], BF16)
            nc.sync.dma_start(wfb_sb[:], wfb[:, :])
            b1c_sb = const.tile([P, 1], F32)
            nc.sync.dma_start(b1c_sb[:], b1c[:, :])
            b2c_sb = const.tile([P, 1], F32)
            nc.sync.dma_start(b2c_sb[:], b2c[:, :])
            blc_sb = const.tile([P, 1], F32)
            nc.sync.dma_start(blc_sb[:], blc[:, :])
            b2fr_sb = const.tile([1, SOUT], F32)
            nc.sync.dma_start(b2fr_sb[:], b2fr[:, :])
            ones1_sb = const.tile([1, P], F32)
            nc.sync.dma_start(ones1_sb[:], ones1[:, :])
            idn_sb = const.tile([P, P], BF16)
            nc.sync.dma_start(idn_sb[:], idn[:, :])
            amat_sb = aggp.tile([P, NW, NDP], BF16)
            nc.sync.dma_start(amat_sb[:], amat[:, :, :])
            apm_sb = aggp.tile([P, NW, P], BF16)
            nc.sync.dma_start(apm_sb[:], apm[:, :, :])
            ma_sb, mb_sb = [], []
            for q in range(4):
                ta = const.tile([P, QW], BF16, tag=f"maq{q}")
                nc.sync.dma_start(ta[:], ma[:, q * QW : (q + 1) * QW])
                ma_sb.append(ta)
                tb = const.tile([P, QW], BF16, tag=f"mbq{q}")
                nc.sync.dma_start(tb[:], mb[:, q * QW : (q + 1) * QW])
                mb_sb.append(tb)

            h2t = persist.tile([P, NW * P], BF16)   # encoder out [d, node]
            h2n = persist.tile([P, NW, P], BF16)    # transposed   [node, d]
            xt = persist.tile([P, NW, P], BF16)     # finalX       [d, node]
            u_sb = persist.tile([P, NW, SOUT], BF16)  # U = finalX@Wfa [node, se]
            v_sb = persist.tile([P, NW, SOUT], BF16)  # V = finalX@Wfb [node, se]

            # ---- phase 1: encoder MLP (replicated) ----
            with tc.tile_pool(name="encps", bufs=2, space=bass.MemorySpace.PSUM) as encps, \
                 tc.tile_pool(name="trps", bufs=2, space=bass.MemorySpace.PSUM) as trps:
                h1t = encp.tile([P, ND], BF16)
                nc.vector.memset(h2t[:, ND:], 0.0)
                for c0, cw in ((0, 512), (512, ND - 512)):
                    ph = encps.tile([P, 512], F32, tag="ph")
                    for k in range(KF):
                        nc.tensor.matmul(
                            ph[:, :cw],
                            w1l_sb[:, k, :],
                            dft_sb[k // 2][:, k % 2, c0 : c0 + cw],
                            start=(k == 0),
                            stop=(k == KF - 1),
                        )
                    nc.scalar.activation(
                        h1t[:, c0 : c0 + cw], ph[:, :cw],
                        mybir.ActivationFunctionType.Relu, bias=b1c_sb[:],
                    )
                for c0, cw in ((0, 512), (512, ND - 512)):
                    ph = encps.tile([P, 512], F32, tag="ph")
                    nc.tensor.matmul(ph[:, :cw], w2_sb[:], h1t[:, c0 : c0 + cw])
                    nc.scalar.activation(
                        h2t[:, c0 : c0 + cw], ph[:, :cw],
                        mybir.ActivationFunctionType.Relu, bias=b2c_sb[:],
                    )
                # h2 windows transposed to [node, d] (lhsT for aggregation)
                for w in range(NW):
                    pt = trps.tile([P, P], BF16, tag="pt")
                    nc.tensor.transpose(pt[:], h2t[:, w * P : (w + 1) * P], idn_sb[:])
                    nc.scalar.copy(h2n[:, w, :], pt[:])
            encp.release()

            # ---- phase 2: aggregation + SAGE + U/V (replicated) ----
            with tc.tile_pool(name="gnn", bufs=2) as gnnp, \
                 tc.tile_pool(name="aggps", bufs=2, space=bass.MemorySpace.PSUM) as aggps, \
                 tc.tile_pool(name="smps", bufs=2, space=bass.MemorySpace.PSUM) as smps, \
                 tc.tile_pool(name="uvps", bufs=1, space=bass.MemorySpace.PSUM) as uvps:
                # decoder bias folded into U: every pair reads exactly one U
                # row, so U += B (broadcast to all node rows) replaces the
                # per-chunk bias add in the decoder copies
                bbc = gnnp.tile([P, SOUT], BF16, tag="bbc")
                pb = uvps.tile([P, SOUT], F32, tag="pb")
                nc.tensor.matmul(pb[:], ones1_sb[:], b2fr_sb[:])
                nc.scalar.copy(bbc[:], pb[:])
                for w in range(NW):
                    pagg = aggps.tile([P, P], F32, tag="pagg")
                    for sw in range(NW):
                        nc.tensor.matmul(
                            pagg[:], h2n[:, sw, :],
                            amat_sb[:, sw, w * P : (w + 1) * P],
                            start=(sw == 0), stop=False,
                        )
                    nc.tensor.matmul(pagg[:], apm_sb[:, w, :], idn_sb[:],
                                     start=False, stop=True)
                    meant = gnnp.tile([P, P], BF16, tag="meant")
                    nc.scalar.copy(meant[:], pagg[:])

                    px = smps.tile([P, P], F32, tag="px")
                    nc.tensor.matmul(px[:], wl_sb[:], meant[:], start=True, stop=False)
                    nc.tensor.matmul(px[:], wr_sb[:], h2t[:, w * P : (w + 1) * P],
                                     start=False, stop=True)
                    nc.scalar.activation(
                        xt[:, w, :], px[:],
                        mybir.ActivationFunctionType.Relu, bias=blc_sb[:],
                    )
                    pu = uvps.tile([P, SOUT], F32, tag="pu")
                    nc.tensor.matmul(pu[:], xt[:, w, :], wfa_sb[:])
                    nc.vector.tensor_tensor(
                        u_sb[:, w, :], pu[:], bbc[:], mybir.AluOpType.add
                    )
                    pv = uvps.tile([P, SOUT], F32, tag="pv")
                    nc.tensor.matmul(pv[:], xt[:, w, :], wfb_sb[:])
                    nc.vector.tensor_copy(v_sb[:, w, :], pv[:])
            aggp.release()

            # ---- phase 3: decoder (sharded over cores) ----
            with tc.tile_pool(name="dec", bufs=2) as decp, \
                 tc.tile_pool(name="decps", bufs=4, space=bass.MemorySpace.PSUM) as decps:
                pe = max(o + n for _, _, o, n in chunks)
                for mt in range(MT):
                    stage = decp.tile([P, TCPP], BF16, tag="stage")
                    sent = 0
                    for ci, (wa, wb, off, n) in enumerate(chunks):
                        q, lo = off // QW, off % QW
                        assert lo + n <= QW
                        po = decps.tile([P, 512], F32, tag="po")
                        nc.tensor.matmul(
                            po[:MROW, :n],
                            u_sb[:, wa, mt * MROW : (mt + 1) * MROW],
                            ma_sb[q][:, lo : lo + n],
                            start=True, stop=False,
                        )
                        nc.tensor.matmul(
                            po[:MROW, :n],
                            v_sb[:, wb, mt * MROW : (mt + 1) * MROW],
                            mb_sb[q][:, lo : lo + n],
                            start=False, stop=True,
                        )
                        if ci % 2 == 0:
                            nc.vector.tensor_copy(
                                stage[:MROW, off : off + n], po[:MROW, :n]
                            )
                        else:
                            nc.scalar.copy(
                                stage[:MROW, off : off + n], po[:MROW, :n]
                            )
                        # flush each completed quarter with a large SWDGE DMA
                        # (HWDGE SBUF->DRAM descriptors land on only 5 SDMA
                        # engines; gpsimd spreads across all 16)
                        end = off + n
                        if end == pe or end % QW == 0:
                            nc.gpsimd.dma_start(out[mt, :, sent:end],
                                                stage[:MROW, sent:end])
                            sent = end

    nc.compile()
    return nc


def _prepare(inputs):
    """Host-side preprocessing: weight fusion/casts, adjacency + protein
    aggregation matrices, per-core sorted pair one-hots."""
    dF = np.asarray(inputs["drugFeatures"], np.float32)
    ei = np.asarray(inputs["edge_index"])
    tpl = np.asarray(inputs["tpl"])
    samp = np.asarray(inputs["sampleSes"]).astype(np.int64)
    W1 = np.asarray(inputs["W1"], np.float32)
    b1 = np.asarray(inputs["b1"], np.float32)
    W2 = np.asarray(inputs["W2"], np.float32)
    b2 = np.asarray(inputs["b2"], np.float32)
    prot = np.asarray(inputs["proteinEmb"], np.float32)
    sageWl = np.asarray(inputs["sageWl"], np.float32)
    sageBl = np.asarray(inputs["sageBl"], np.float32)
    sageWr = np.asarray(inputs["sageWr"], np.float32)
    outW1 = np.asarray(inputs["outW1"], np.float32)
    outB1 = np.asarray(inputs["outB1"], np.float32)
    outW2 = np.asarray(inputs["outW2"], np.float32)
    outB2 = np.asarray(inputs["outB2"], np.float32)

    # ---- edges: keep dst < ND; fold 1/max(cnt,1) into A and aggprot ----
    src = ei[0].astype(np.int64)
    dst = ei[1].astype(np.int64)
    keep = dst < ND
    src = src[keep]
    dst = dst[keep]
    cnt = np.bincount(dst, minlength=NDP).astype(np.float32)[:NDP]
    rr = 1.0 / np.maximum(cnt, 1.0)

    isdrug = src < ND
    sd, dd = src[isdrug], dst[isdrug]
    A = np.zeros((NDP, NDP), np.float32)          # [src, dst] edge counts
    np.add.at(A, (sd, dd), 1.0)
    A *= rr[None, :]
    amat = _bf16(A.reshape(NW, P, NDP).transpose(1, 0, 2))

    sp, dp = src[~isdrug] - ND, dst[~isdrug]
    ap = np.zeros((NDP, D), np.float32)           # [dst, d] protein sums
    np.add.at(ap, dp, prot[sp])
    ap *= rr[:, None]
    apm = _bf16(ap.reshape(NW, P, D).transpose(1, 0, 2))

    # ---- fused decoder weights ----
    W2s = outW2[:, samp]                     # [128, 500]
    Wf = outW1 @ W2s                         # [256, 500]
    B2f = outB1 @ W2s + outB2[samp]          # [500]

    common = dict(
        dft=_bf16(dF.T.reshape(KF, P, ND).transpose(1, 0, 2)),
        w1l=_bf16(W1.reshape(KF, P, P).transpose(1, 0, 2)),
        w2=_bf16(W2),
        wl=_bf16(sageWl),
        wr=_bf16(sageWr),
        wfa=_bf16(Wf[:P, :]),
        wfb=_bf16(Wf[P:, :]),
        b1c=b1.reshape(P, 1).astype(np.float32),
        b2c=b2.reshape(P, 1).astype(np.float32),
        blc=sageBl.reshape(P, 1).astype(np.float32),
        b2fr=B2f.reshape(1, SOUT).astype(np.float32),
        ones1=np.ones((1, P), np.float32),
        idn=_bf16(np.eye(P)),
        amat=amat,
        apm=apm,
    )

    # ---- global pair sort into 64 (wa, wb) buckets, dealt across cores ----
    # All T pairs sort by bucket; within a bucket, element r goes to core
    # r % 8 at packed column offs[bucket] + r // 8.  Per-core bucket sizes
    # then differ by <=1, so one shared chunk layout serves every core.
    a_all = tpl[:, 0].astype(np.int64)
    b_all = tpl[:, 1].astype(np.int64)
    key = (a_all >> 7) * NW + (b_all >> 7)
    order = np.argsort(key, kind="stable")
    ksort = key[order]
    sizes = np.bincount(ksort, minlength=NW * NW)
    starts = np.cumsum(sizes) - sizes
    rank = np.arange(T) - np.repeat(starts, sizes)
    core = rank % NCORES
    bucket_pad = ((sizes + NCORES - 1) // NCORES + 3) // 4 * 4
    offs = np.zeros(NW * NW + 1, np.int64)
    offs[1:] = np.cumsum(bucket_pad)
    assert offs[-1] <= TCPP, (offs[-1], TCPP)
    col = offs[ksort] + rank // NCORES
    chunks = []
    for bi in range(NW * NW):
        wa, wb = bi // NW, bi % NW
        o, n = int(offs[bi]), int(bucket_pad[bi])
        while n:
            # split at 512 and at M-upload quarter boundaries
            step = min(n, 512, QW - o % QW)
            chunks.append((wa, wb, o, step))
            o += step
            n -= step
    assert len(chunks) <= NCHMAX, len(chunks)

    a_sorted = a_all[order]
    b_sorted = b_all[order]
    gcols = np.empty(T, np.int64)
    gcols[order] = core * TCPP + col
    ma_list, mb_list = [], []
    for c in range(NCORES):
        m = core == c
        Ma = np.zeros((P, TCPP), ml_dtypes.bfloat16)
        Mb = np.zeros((P, TCPP), ml_dtypes.bfloat16)
        Ma[a_sorted[m] & 127, col[m]] = 1.0
        Mb[b_sorted[m] & 127, col[m]] = 1.0
        ma_list.append(Ma)
        mb_list.append(Mb)
    return common, chunks, ma_list, mb_list, gcols


LAST_RESULTS = None


def kernel(**inputs):
    common, chunks, ma_list, mb_list, gcols = _prepare(inputs)
    nc = _build_program(chunks)

    in_maps = [
        {**common, "ma": ma_list[c], "mb": mb_list[c]} for c in range(NCORES)
    ]

    if os.environ.get("BASS_SIM"):
        from concourse.bass_interp import CoreSim

        sim = CoreSim(nc)
        for k, v in in_maps[0].items():
            sim.tensor(k)[:] = v
        sim.simulate()
        outs = [np.array(sim.tensor("out"))]
        outs = outs * NCORES  # sim runs core 0 only; replicate for shape
    else:
        from concourse.bass_utils import run_bass_kernel_spmd

        res = run_bass_kernel_spmd(
            nc, in_maps, list(range(NCORES)),
            tmpdir=os.environ.get("BASS_TRACE_DIR") or None,
        )
        global LAST_RESULTS
        LAST_RESULTS = res
        outs = [res.results[c]["out"] for c in range(NCORES)]

    full = np.concatenate(
        [np.asarray(o).reshape(SOUT, TCPP) for o in outs], axis=1
    )  # [500, 8*TCPP] bf16, packed-sorted columns
    fullT = np.ascontiguousarray(full.T)
    return fullT[gcols].astype(np.float32).reshape(-1)


if __name__ == "__main__":
    rng = np.random.default_rng(0)
    fake = dict(
        drugFeatures=rng.standard_normal((ND, F), dtype=np.float32),
        edge_index=rng.integers(0, 21000, (2, 640000)),
        tpl=rng.integers(0, ND, (T, 2)),
        sampleSes=rng.integers(0, 964, (SOUT,)),
        W1=rng.standard_normal((F, D), dtype=np.float32) * 0.02,
        b1=np.zeros(D, np.float32),
        W2=rng.standard_normal((D, D), dtype=np.float32) * 0.05,
        b2=np.zeros(D, np.float32),
        proteinEmb=rng.uniform(0.001, 0.3, (20000, D)).astype(np.float32),
        sageWl=rng.standard_normal((D, D), dtype=np.float32) * 0.05,
        sageBl=np.zeros(D, np.float32),
        sageWr=rng.standard_normal((D, D), dtype=np.float32) * 0.05,
        outW1=rng.standard_normal((2 * D, D), dtype=np.float32) * 0.05,
        outB1=np.zeros(D, np.float32),
        outW2=rng.standard_normal((D, 964), dtype=np.float32) * 0.05,
        outB2=np.zeros(964, np.float32),
    )
    out = kernel(**fake)
    print(out.shape, out.dtype)
